# revision 1
# baseline (speedup 1.0000x reference)
"""CrossFusion block on 8 TRN2 NeuronCores.

Data-parallel over batch: 64 batches -> 8 cores x 8 batches.
All activations kept feature-major ("X.T" layout, feature dim on SBUF
partitions) so every matmul chains without any on-device transposes; all
layout transposition is done host-side while sharding.
Matmuls run in bf16 with f32 PSUM accumulation; layernorm statistics are
computed with ones-vector matmuls on the TensorEngine (partition-dim
reductions) and broadcast back to 128 partitions with rank-1 matmuls.
Softmax skips max-subtraction (scores are O(1) here), normalizes via a
PE rank-1 broadcast of 1/rowsum.
"""

import sys

sys.path.insert(0, "/opt/trn_rl_repo")

import numpy as np
import ml_dtypes

import concourse.bass as bass
import concourse.tile as tile
from concourse import bacc, mybir
from concourse import bass_utils

BF16 = ml_dtypes.bfloat16

B, LC, LL, LLAMA_DIM, DIM, HEADS = 64, 77, 256, 2048, 768, 8
HEAD_DIM = DIM // HEADS          # 96
SCALE = HEAD_DIM ** -0.5
FF = 4 * DIM                     # 3072
NCORES = 8
BPC = B // NCORES                # batches per core = 8
TQ = BPC * LL                    # llm tokens per core = 2048
TK = BPC * LC                    # clip tokens per core = 616
KT_D = DIM // 128                # 6
KT_L = LLAMA_DIM // 128          # 16
KT_F = FF // 128                 # 24
EPS = 1e-5

F32 = mybir.dt.float32
BF = mybir.dt.bfloat16
AF = mybir.ActivationFunctionType
OP = mybir.AluOpType

# packed param tile column offsets (all f32, [128, PP_COLS])
PP_PROJB, PP_OB, PP_F2B = 0, 6, 12
PP_QNG, PP_QNB, PP_KVG, PP_KVB, PP_NG, PP_NB = 18, 24, 30, 36, 42, 48
PP_F1B = 54          # 24 cols
PP_QB = 78           # 8 cols (rows 0..95)
PP_KB = 86           # 8 cols (rows 0..95)
PP_EPS = 94
PP_COLS = 96


def _ln_stats_chunk(nc, pools, x_sb, n_kt, c0, cw, a_row, b_row, ptmp, psum):
    """Emit LN stats for columns [c0, c0+cw) of feature-major x_sb."""
    ones128 = pools["ones128"]
    ps_s = psum.tile([128, 512], F32, tag="ps", name=f"ps_s{c0}")
    ps_q = psum.tile([128, 512], F32, tag="ps", name=f"ps_q{c0}")
    for kt in range(n_kt):
        xs = x_sb[:, kt, c0:c0 + cw]
        sq = ptmp.tile([128, 512], BF, tag="sq")
        nc.scalar.activation(out=sq[:, :cw], in_=xs, func=AF.Square)
        nc.tensor.matmul(ps_s[:1, :cw], ones128, xs,
                         start=(kt == 0), stop=(kt == n_kt - 1))
        nc.tensor.matmul(ps_q[:1, :cw], ones128, sq[:, :cw],
                         start=(kt == 0), stop=(kt == n_kt - 1))
    # rs slices: [0:512) m, [512:1024) mq, [1024:1536) var/std/inv
    rs = ptmp.tile([1, 1536], F32, tag="rs")
    m = rs[:, 0:cw]
    mq = rs[:, 512:512 + cw]
    c = rs[:, 1024:1024 + cw]
    nc.scalar.activation(out=m, in_=ps_s[:1, :cw], func=AF.Copy,
                         scale=1.0 / DIM)
    nc.scalar.activation(out=mq, in_=ps_q[:1, :cw], func=AF.Copy,
                         scale=1.0 / DIM)
    nc.vector.tensor_tensor(out=c, in0=m, in1=m, op=OP.mult)
    nc.vector.tensor_tensor(out=c, in0=mq, in1=c, op=OP.subtract)
    nc.scalar.activation(out=c, in_=c, func=AF.Sqrt, bias=pools["eps1"])
    nc.vector.reciprocal(out=c, in_=c)
    nc.vector.tensor_copy(out=a_row[:, c0:c0 + cw], in_=c)
    nc.vector.tensor_scalar(out=m, in0=m, scalar1=-1.0, scalar2=None,
                            op0=OP.mult)
    nc.vector.tensor_tensor(out=b_row[:, c0:c0 + cw], in0=m, in1=c,
                            op=OP.mult)


def _ln_stats(nc, pools, x_sb, n_kt, ncols, chunk, ptmp, psum, prow):
    a_row = prow.tile([1, TQ], BF, tag="a_row")
    b_row = prow.tile([1, TQ], BF, tag="b_row")
    for ci in range((ncols + chunk - 1) // chunk):
        c0 = ci * chunk
        cw = min(chunk, ncols - c0)
        _ln_stats_chunk(nc, pools, x_sb, n_kt, c0, cw, a_row, b_row,
                        ptmp, psum)
    return a_row, b_row


def _ln_norm_chunk(nc, pools, x_view, out_view, n_kt, cw, a_row, b_row, c0,
                   g_sb, bvec_sb, psum):
    """Normalize one column chunk: out = (x*A + B)*g + b (feature-major)."""
    ones1 = pools["ones1"]
    uniq = f"{c0}_{x_view.tensor.name}"
    ps_a = psum.tile([128, 512], F32, tag="ps", name=f"ps_a_{uniq}")
    ps_b = psum.tile([128, 512], F32, tag="ps", name=f"ps_b_{uniq}")
    nc.tensor.matmul(ps_a[:, :cw], ones1, a_row[:, c0:c0 + cw])
    nc.tensor.matmul(ps_b[:, :cw], ones1, b_row[:, c0:c0 + cw])
    for kt in range(n_kt):
        nc.vector.tensor_tensor(out=out_view[:, kt, :cw],
                                in0=x_view[:, kt, :cw],
                                in1=ps_a[:, :cw], op=OP.mult)
        nc.vector.tensor_tensor(out=out_view[:, kt, :cw],
                                in0=out_view[:, kt, :cw],
                                in1=ps_b[:, :cw], op=OP.add)
        nc.gpsimd.tensor_scalar(out=out_view[:, kt, :cw],
                                in0=out_view[:, kt, :cw],
                                scalar1=g_sb[:, kt:kt + 1],
                                scalar2=bvec_sb[:, kt:kt + 1],
                                op0=OP.mult, op1=OP.add)


def build_nc():
    nc = bacc.Bacc("TRN2", target_bir_lowering=False, debug=False)

    embT = nc.dram_tensor("embT", (KT_L, 128, TQ), BF, kind="ExternalInput")
    clipT = nc.dram_tensor("clipT", (KT_D, 128, TK), BF, kind="ExternalInput")
    wprojT = nc.dram_tensor("wprojT", (KT_L, 128, DIM), BF, kind="ExternalInput")
    wqT = nc.dram_tensor("wqT", (KT_D, 128, DIM), BF, kind="ExternalInput")
    wkT = nc.dram_tensor("wkT", (KT_D, 128, DIM), BF, kind="ExternalInput")
    wvT = nc.dram_tensor("wvT", (KT_D, 128, DIM), BF, kind="ExternalInput")
    woT = nc.dram_tensor("woT", (HEAD_DIM, HEADS, DIM), BF, kind="ExternalInput")
    f1T = nc.dram_tensor("f1T", (KT_D, 128, FF), BF, kind="ExternalInput")
    f2T = nc.dram_tensor("f2T", (KT_F, 128, DIM), BF, kind="ExternalInput")
    pp = nc.dram_tensor("pp", (128, PP_COLS), F32, kind="ExternalInput")
    vb = nc.dram_tensor("vb", (1, DIM), BF, kind="ExternalInput")
    outT = nc.dram_tensor("outT", (KT_D, 128, TQ), F32, kind="ExternalOutput")

    with tile.TileContext(nc) as tc:
        from contextlib import ExitStack
        with ExitStack() as stk:
            pw = stk.enter_context(tc.tile_pool(name="pw", bufs=1))
            pact = stk.enter_context(tc.tile_pool(name="pact", bufs=1))
            prow = stk.enter_context(tc.tile_pool(name="prow", bufs=1))
            ptmp = stk.enter_context(tc.tile_pool(name="ptmp", bufs=2))
            psum = stk.enter_context(
                tc.tile_pool(name="psum", bufs=8, space="PSUM"))

            ones_sq = pw.tile([128, 128], BF, tag="ones")
            nc.vector.memset(ones_sq, 1.0)
            ones128 = ones_sq[:, 0:1]
            ones1 = ones_sq[0:1, :]
            pp_sb = pw.tile([128, PP_COLS], F32, tag="pp")
            nc.sync.dma_start(out=pp_sb, in_=pp.ap())
            vb_sb = pw.tile([1, DIM], BF, tag="vb")
            nc.sync.dma_start(out=vb_sb, in_=vb.ap())
            pools = {"ones128": ones128, "ones1": ones1,
                     "eps1": pp_sb[:1, PP_EPS:PP_EPS + 1]}

            def ppc(col, n=1, rows=128):
                return pp_sb[:rows, col:col + n]

            def load3(pool, dram, shape, name):
                t = pool.tile(list(shape), dram.dtype, tag=name)
                for k in range(shape[1]):
                    nc.sync.dma_start(out=t[:, k, :], in_=dram.ap()[k])
                return t

            llm_sb = pact.tile([128, KT_D, TQ], BF, tag="llm")     # llm.T/llm2.T
            k_sb = pact.tile([HEAD_DIM, HEADS, TK], BF, tag="k")   # k.T hd-major
            v_sb = pact.tile([LC, BPC, DIM], BF, tag="v")          # v tok-major

            # ====== Stage P+C: clip path first (fills ACT/DVE while proj
            # matmuls dominate PE), proj with LN_kv stats interleaved ======
            a_kv = prow.tile([1, TQ], BF, tag="a_row", name="a_kv")
            b_kv = prow.tile([1, TQ], BF, tag="b_row", name="b_kv")
            with tc.tile_pool(name="pkvw", bufs=1) as pkvw, \
                 tc.tile_pool(name="pclip", bufs=1) as pclip:
                clip_sb = load3(pclip, clipT, (128, KT_D, TK), "clip")
                clipn_sb = pclip.tile([128, KT_D, TK], BF, tag="clipn")

                a_c = prow.tile([1, TK], BF, tag="ac_row", name="a_c")
                b_c = prow.tile([1, TK], BF, tag="bc_row", name="b_c")
                for ci in range(2):
                    _ln_stats_chunk(nc, pools, clip_sb, KT_D, ci * 308, 308,
                                    a_c, b_c, ptmp, psum)
                for ci in range(2):
                    c0 = ci * 308
                    _ln_norm_chunk(nc, pools, clip_sb[:, :, c0:c0 + 308],
                                   clipn_sb[:, :, c0:c0 + 308], KT_D, 308,
                                   a_c, b_c, c0, ppc(PP_QNG, KT_D),
                                   ppc(PP_QNB, KT_D), psum)

                # ---- proj (llm_embed @ Wp.T), LN_kv stats per chunk ----
                with tc.tile_pool(name="pwproj", bufs=1) as pwproj, \
                     tc.tile_pool(name="pemb", bufs=2) as pemb:
                    wp_sb = load3(pwproj, wprojT, (128, KT_L, DIM), "wproj")
                    NCH = 512
                    for ci in range(TQ // NCH):
                        c0 = ci * NCH
                        emb_c = pemb.tile([128, KT_L, NCH], BF, tag="emb_c")
                        for kt in range(KT_L):
                            nc.sync.dma_start(out=emb_c[:, kt, :],
                                              in_=embT.ap()[kt, :, c0:c0 + NCH])
                        for mt in range(KT_D):
                            ps = psum.tile([128, 512], F32, tag="ps")
                            for kt in range(KT_L):
                                nc.tensor.matmul(
                                    ps, wp_sb[:, kt, mt * 128:(mt + 1) * 128],
                                    emb_c[:, kt, :],
                                    start=(kt == 0), stop=(kt == KT_L - 1))
                            nc.scalar.activation(
                                out=llm_sb[:, mt, c0:c0 + NCH], in_=ps,
                                func=AF.Identity, bias=ppc(PP_PROJB + mt))
                        _ln_stats_chunk(nc, pools, llm_sb, KT_D, c0, NCH,
                                        a_kv, b_kv, ptmp, psum)

                wk_sb = load3(pkvw, wkT, (128, KT_D, DIM), "wk")
                wv_sb = load3(pkvw, wvT, (128, KT_D, DIM), "wv")

                # k.T head-major [96, h, 616]
                for h in range(HEADS):
                    for ci in range(2):
                        c0 = ci * 308
                        ps = psum.tile([128, 512], F32, tag="ps")
                        for kt in range(KT_D):
                            nc.tensor.matmul(
                                ps[:HEAD_DIM, :308],
                                wk_sb[:, kt, h * 96:(h + 1) * 96],
                                clipn_sb[:, kt, c0:c0 + 308],
                                start=(kt == 0), stop=(kt == KT_D - 1))
                        nc.scalar.activation(
                            out=k_sb[:, h, c0:c0 + 308],
                            in_=ps[:HEAD_DIM, :308],
                            func=AF.Identity, bias=ppc(PP_KB + h, rows=96))

                # v token-major [77, b, 768] (bias added via rank-1 matmul)
                for b in range(BPC):
                    for ci in range(2):
                        c0 = ci * 384
                        ps = psum.tile([128, 512], F32, tag="ps")
                        for kt in range(KT_D):
                            nc.tensor.matmul(
                                ps[:LC, :384],
                                clipn_sb[:, kt, b * LC:(b + 1) * LC],
                                wv_sb[:, kt, c0:c0 + 384],
                                start=(kt == 0), stop=False)
                        nc.tensor.matmul(ps[:LC, :384], ones1[:, :LC],
                                         vb_sb[:, c0:c0 + 384],
                                         start=False, stop=True)
                        nc.scalar.activation(out=v_sb[:, b, c0:c0 + 384],
                                             in_=ps[:LC, :384],
                                             func=AF.Identity)

            # ============= Stage M: q + attention + o (lnn lookahead) ======
            a_2 = prow.tile([1, TQ], BF, tag="a2_row", name="a_2")
            b_2 = prow.tile([1, TQ], BF, tag="b2_row", name="b_2")

            with tc.tile_pool(name="pqw", bufs=1) as pqw, \
                 tc.tile_pool(name="pmid", bufs=3) as pmid, \
                 tc.tile_pool(name="patn", bufs=8) as patn:
                wq_sb = load3(pqw, wqT, (128, KT_D, DIM), "wq")
                wo_sb = pqw.tile([HEAD_DIM, HEADS, DIM], BF, tag="wo")
                for h in range(HEADS):
                    nc.sync.dma_start(out=wo_sb[:, h, :], in_=woT.ap()[:, h, :])

                lnns = {}

                def emit_lnn(bb):
                    t = pmid.tile([128, KT_D, LL], BF, tag="lnn",
                                  name=f"lnn{bb}")
                    cc = bb * LL
                    _ln_norm_chunk(nc, pools, llm_sb[:, :, cc:cc + LL],
                                   t, KT_D, LL, a_kv, b_kv, cc,
                                   ppc(PP_KVG, KT_D), ppc(PP_KVB, KT_D), psum)
                    lnns[bb] = t

                def emit_q(bb):
                    lnn = lnns.pop(bb)
                    t = pmid.tile([HEAD_DIM, HEADS, LL], BF, tag="q_c",
                                  name=f"q_c{bb}")
                    for h in range(HEADS):
                        ps = psum.tile([128, 512], F32, tag="ps",
                                       name=f"ps_qp_{bb}_{h}")
                        for kt in range(KT_D):
                            nc.tensor.matmul(
                                ps[:HEAD_DIM, :LL],
                                wq_sb[:, kt, h * 96:(h + 1) * 96],
                                lnn[:, kt, :],
                                start=(kt == 0), stop=(kt == KT_D - 1))
                        nc.scalar.activation(
                            out=t[:, h, :], in_=ps[:HEAD_DIM, :LL],
                            func=AF.Identity, bias=ppc(PP_QB + h, rows=96))
                    return t

                emit_lnn(0)
                for b in range(BPC):
                    c0 = b * LL
                    q_c = emit_q(b)
                    if b + 1 < BPC:
                        emit_lnn(b + 1)

                    # attention, op-type-major over groups of 4 heads so the
                    # per-head engine chains pipeline instead of serializing.
                    # one psum bank per head packs: scores [0:77, 0:256],
                    # softmax-sum row [96:97, 0:256], 1/sum bcast
                    # [0:77, 256:512] -- the chain orders accesses anyway.
                    # ex tile row 77 holds 1/sum (partition-disjoint).
                    # attention, op-type-major over groups of 4 heads so the
                    # per-head engine chains pipeline. Per head, 2 psum banks
                    # (all accesses ordered by the softmax chain itself):
                    #   ps1: scores [0:77, 0:256] | att@v out [0:96, 256:512]
                    #   ps2: softmax sum [0:1, 0:256] | 1/sum bcast [0:77, 256:512]
                    ao_c = pmid.tile([HEAD_DIM, HEADS, LL], BF, tag="ao_c")
                    for g in range(2):
                        hs = list(range(4 * g, 4 * g + 4))
                        ps1, ps2 = {}, {}
                        for h in hs:
                            ps1[h] = psum.tile([128, 512], F32, tag="ps", name=f"ps1_{b}_{h}")
                            nc.tensor.matmul(ps1[h][:LC, :LL],
                                             k_sb[:, h, b * LC:(b + 1) * LC],
                                             q_c[:, h, :])
                        ex = {}
                        for h in hs:
                            ex[h] = patn.tile([LC, LL], BF, tag="ex", name=f"ex_{b}_{h}")
                            nc.scalar.activation(out=ex[h],
                                                 in_=ps1[h][:LC, :LL],
                                                 func=AF.Exp, scale=SCALE)
                        for h in hs:
                            ps2[h] = psum.tile([128, 512], F32, tag="ps", name=f"ps2_{b}_{h}")
                            nc.tensor.matmul(ps2[h][:1, :LL],
                                             ones128[:LC, :], ex[h])
                        inv = {}
                        for h in hs:
                            inv[h] = patn.tile([1, LL], BF, tag="inv", name=f"inv_{b}_{h}")
                            with nc.allow_low_precision("softmax 1/sum bf16"):
                                nc.vector.reciprocal(out=inv[h],
                                                     in_=ps2[h][:1, :LL])
                        for h in hs:
                            nc.tensor.matmul(ps2[h][:LC, LL:2 * LL],
                                             ones1[:, :LC], inv[h])
                        for h in hs:
                            nc.vector.tensor_tensor(out=ex[h], in0=ex[h],
                                                    in1=ps2[h][:LC, LL:2 * LL],
                                                    op=OP.mult)
                        for h in hs:
                            nc.tensor.matmul(
                                ps1[h][:HEAD_DIM, LL:2 * LL],
                                v_sb[:, b, h * 96:(h + 1) * 96], ex[h])
                        for h in hs:
                            nc.scalar.activation(out=ao_c[:, h, :],
                                                 in_=ps1[h][:HEAD_DIM, LL:2 * LL],
                                                 func=AF.Identity)

                    # o-proj + bias + residual (in-place: llm becomes llm2)
                    for mt in range(KT_D):
                        ps = psum.tile([128, 512], F32, tag="ps")
                        for h in range(HEADS):
                            nc.tensor.matmul(
                                ps[:, :LL],
                                wo_sb[:, h, mt * 128:(mt + 1) * 128],
                                ao_c[:, h, :],
                                start=(h == 0), stop=(h == HEADS - 1))
                        nc.vector.scalar_tensor_tensor(
                            out=llm_sb[:, mt, c0:c0 + LL],
                            in0=ps[:, :LL],
                            scalar=ppc(PP_OB + mt),
                            in1=llm_sb[:, mt, c0:c0 + LL],
                            op0=OP.add, op1=OP.add)
                    if b % 2 == 1:
                        # LN2 stats for the 512-token chunk just completed
                        _ln_stats_chunk(nc, pools, llm_sb, KT_D,
                                        (b - 1) * LL, 2 * LL,
                                        a_2, b_2, ptmp, psum)

            # ================= Stage F: LN2 + FFN ==========================
            with tc.tile_pool(name="pfw", bufs=1) as pfw, \
                 tc.tile_pool(name="pffn", bufs=2) as pffn, \
                 tc.tile_pool(name="pfc", bufs=1) as pfc, \
                 tc.tile_pool(name="pout", bufs=2) as pout:
                f1_sb = load3(pfw, f1T, (128, KT_D, FF), "f1")
                f2_sb = load3(pfw, f2T, (128, KT_F, DIM), "f2")
                NCH = 512
                NFC = TQ // NCH
                h_cs = {}

                def emit_h(cc):
                    t = pffn.tile([128, KT_D, NCH], BF, tag="h_c",
                                  name=f"h_c{cc}")
                    _ln_norm_chunk(nc, pools,
                                   llm_sb[:, :, cc * NCH:(cc + 1) * NCH],
                                   t, KT_D, NCH, a_2, b_2, cc * NCH,
                                   ppc(PP_NG, KT_D), ppc(PP_NB, KT_D), psum)
                    h_cs[cc] = t

                emit_h(0)
                for ci in range(NFC):
                    c0 = ci * NCH
                    h_c = h_cs.pop(ci)
                    if ci + 1 < NFC:
                        emit_h(ci + 1)
                    f_c = pfc.tile([128, KT_F, NCH], BF, tag="f_c")
                    for mt in range(KT_F):
                        ps = psum.tile([128, 512], F32, tag="ps")
                        for kt in range(KT_D):
                            nc.tensor.matmul(
                                ps[:, :NCH],
                                f1_sb[:, kt, mt * 128:(mt + 1) * 128],
                                h_c[:, kt, :],
                                start=(kt == 0), stop=(kt == KT_D - 1))
                        nc.scalar.activation(
                            out=f_c[:, mt, :], in_=ps[:, :NCH],
                            func=AF.Gelu_apprx_sigmoid,
                            bias=ppc(PP_F1B + mt))
                    for mt in range(KT_D):
                        ps = psum.tile([128, 512], F32, tag="ps")
                        for kt in range(KT_F):
                            nc.tensor.matmul(
                                ps[:, :NCH],
                                f2_sb[:, kt, mt * 128:(mt + 1) * 128],
                                f_c[:, kt, :],
                                start=(kt == 0), stop=(kt == KT_F - 1))
                        o_c = pout.tile([128, NCH], F32, tag="o_c")
                        nc.vector.scalar_tensor_tensor(
                            out=o_c, in0=ps[:, :NCH], scalar=ppc(PP_F2B + mt),
                            in1=llm_sb[:, mt, c0:c0 + NCH],
                            op0=OP.add, op1=OP.add)
                        nc.sync.dma_start(out=outT.ap()[mt, :, c0:c0 + NCH],
                                          in_=o_c)

    nc.compile()
    return nc


_NC_CACHE = {}


def _get_nc():
    if "nc" not in _NC_CACHE:
        _NC_CACHE["nc"] = build_nc()
    return _NC_CACHE["nc"]


def _prep_in_maps(inputs):
    f32 = np.float32

    def bf(x):
        return np.ascontiguousarray(x).astype(BF16)

    w = {}
    w["wprojT"] = bf(inputs["llm_proj_w"].astype(f32).T.reshape(KT_L, 128, DIM))
    w["wqT"] = bf(inputs["q_w"].astype(f32).T.reshape(KT_D, 128, DIM))
    w["wkT"] = bf(inputs["k_w"].astype(f32).T.reshape(KT_D, 128, DIM))
    w["wvT"] = bf(inputs["v_w"].astype(f32).T.reshape(KT_D, 128, DIM))
    w["woT"] = bf(np.ascontiguousarray(
        inputs["o_w"].astype(f32).T.reshape(HEADS, HEAD_DIM, DIM)
        .transpose(1, 0, 2)))
    w["f1T"] = bf(inputs["f1_w"].astype(f32).T.reshape(KT_D, 128, FF))
    w["f2T"] = bf(inputs["f2_w"].astype(f32).T.reshape(KT_F, 128, DIM))
    w["vb"] = bf(inputs["v_b"].astype(f32).reshape(1, DIM))

    ppa = np.zeros((128, PP_COLS), dtype=f32)

    def put(col, vec, n):
        ppa[:, col:col + n] = np.asarray(vec, dtype=f32).reshape(n, 128).T

    put(PP_PROJB, inputs["llm_proj_b"], KT_D)
    put(PP_OB, inputs["o_b"], KT_D)
    put(PP_F2B, inputs["f2_b"], KT_D)
    put(PP_QNG, inputs["qn_g"], KT_D)
    put(PP_QNB, inputs["qn_b"], KT_D)
    put(PP_KVG, inputs["kvn_g"], KT_D)
    put(PP_KVB, inputs["kvn_b"], KT_D)
    put(PP_NG, inputs["n_g"], KT_D)
    put(PP_NB, inputs["n_b"], KT_D)
    put(PP_F1B, inputs["f1_b"], KT_F)
    ppa[:HEAD_DIM, PP_QB:PP_QB + HEADS] = np.asarray(
        inputs["q_b"], dtype=f32).reshape(HEADS, HEAD_DIM).T
    ppa[:HEAD_DIM, PP_KB:PP_KB + HEADS] = np.asarray(
        inputs["k_b"], dtype=f32).reshape(HEADS, HEAD_DIM).T
    ppa[:, PP_EPS] = EPS
    w["pp"] = ppa

    clip = np.asarray(inputs["clip_embed"], dtype=f32)
    llm = np.asarray(inputs["llm_embed"], dtype=f32)
    in_maps = []
    for c in range(NCORES):
        cs = slice(c * BPC, (c + 1) * BPC)
        m = dict(w)
        m["embT"] = bf(llm[cs].reshape(TQ, LLAMA_DIM).T.reshape(KT_L, 128, TQ))
        m["clipT"] = bf(clip[cs].reshape(TK, DIM).T.reshape(KT_D, 128, TK))
        in_maps.append(m)
    return in_maps


def run(inputs, trace=False):
    nc = _get_nc()
    in_maps = _prep_in_maps(inputs)
    res = bass_utils.run_bass_kernel_spmd(
        nc, in_maps, core_ids=list(range(NCORES)), trace=trace)
    clip = np.asarray(inputs["clip_embed"], dtype=np.float32)
    llm3 = np.empty((B, LL, DIM), dtype=np.float32)
    for c in range(NCORES):
        yT = res.results[c]["outT"].reshape(DIM, TQ)
        llm3[c * BPC:(c + 1) * BPC] = yT.T.reshape(BPC, LL, DIM)
    out = np.concatenate([clip, llm3], axis=1)
    return out, res


def kernel(**inputs):
    out, _ = run(inputs, trace=False)
    return out



# revision 11
# speedup vs baseline: 1.1351x; 1.1351x over previous
"""CrossFusion block on 8 TRN2 NeuronCores.

Data-parallel over batch: 64 batches -> 8 cores x 8 batches.
Feature-major activations (feature dim on SBUF partitions); all layout
transposition host-side.

v2: all linear layers run fp8(e4m3) DoubleRow matmuls with hi/lo
split-precision compensation: weights are split host-side into
w = (hi + lo)/256 fp8 pairs and both halves are contracted (2x fewer PE
cycles than bf16 in pairs-of-k-tiles DoubleRow mode); the proj layer
additionally compensates the activation side (llm_embed split hi/lo,
3-product chain) because its error feeds the residual stream directly.
LayerNorm gains/biases are folded into the consuming weights host-side,
so on-device LN is xn = (x - mean) * inv_std only. inv_std is computed
as exp(-0.5*ln(var+eps)) on the Activation engine so stages P+M need
only the natural_log_exp activation table (softmax exp shares it) and
stage F needs one switch to the gelu table: 2 table loads total.
PSUM->SBUF copy-outs ride on the Pool engine (tensor_scalar with 1/256
descale + per-partition bias); residual adds on DVE; per-batch proj
matmul chunks are interleaved into the attention loop so the PE stays
busy while softmax runs on ACT/DVE.
"""

import sys

sys.path.insert(0, "/opt/trn_rl_repo")

import numpy as np
import ml_dtypes

import concourse.bass as bass
import concourse.tile as tile
from concourse import bacc, mybir
from concourse import bass_utils

BF16 = ml_dtypes.bfloat16
F8NP = ml_dtypes.float8_e4m3

B, LC, LL, LLAMA_DIM, DIM, HEADS = 64, 77, 256, 2048, 768, 8
HEAD_DIM = DIM // HEADS          # 96
SCALE = HEAD_DIM ** -0.5
FF = 4 * DIM                     # 3072
NCORES = 8
BPC = B // NCORES                # batches per core = 8
TQ = BPC * LL                    # llm tokens per core = 2048
TK = BPC * LC                    # clip tokens per core = 616
KT_D = DIM // 128                # 6
KT_L = LLAMA_DIM // 128          # 16
KT_F = FF // 128                 # 24
EPS = 1e-5
SW = 256.0                       # fp8 weight scale
SE = 16.0                        # fp8 llm_embed scale

F32 = mybir.dt.float32
BF = mybir.dt.bfloat16
F8 = mybir.dt.float8e4
AF = mybir.ActivationFunctionType
OP = mybir.AluOpType
DR = mybir.MatmulPerfMode.DoubleRow

# packed param tile column offsets (all f32, [128, PP_COLS])
PP_PROJB = 0          # 6
PP_F1B = 6            # 24
PP_QB = 30            # 8 cols (rows 0..95)
PP_KB = 38            # 8 cols (rows 0..95)
PP_EPS = 46
PP_COLS = 47


def build_nc():
    nc = bacc.Bacc("TRN2", target_bir_lowering=False, debug=False)

    embH = nc.dram_tensor("embH", (KT_L, 128, TQ), F8, kind="ExternalInput")
    embL = nc.dram_tensor("embL", (KT_L, 128, TQ), F8, kind="ExternalInput")
    clipT = nc.dram_tensor("clipT", (KT_D, 128, TK), BF, kind="ExternalInput")
    wpH = nc.dram_tensor("wpH", (KT_L, 128, DIM), F8, kind="ExternalInput")
    wpL = nc.dram_tensor("wpL", (KT_L, 128, DIM), F8, kind="ExternalInput")
    wqH = nc.dram_tensor("wqH", (KT_D, 128, DIM), F8, kind="ExternalInput")
    wqL = nc.dram_tensor("wqL", (KT_D, 128, DIM), F8, kind="ExternalInput")
    wkH = nc.dram_tensor("wkH", (KT_D, 128, DIM), F8, kind="ExternalInput")
    wkL = nc.dram_tensor("wkL", (KT_D, 128, DIM), F8, kind="ExternalInput")
    wvH = nc.dram_tensor("wvH", (KT_D, 128, DIM), F8, kind="ExternalInput")
    wvL = nc.dram_tensor("wvL", (KT_D, 128, DIM), F8, kind="ExternalInput")
    woH = nc.dram_tensor("woH", (HEAD_DIM, HEADS, DIM), F8, kind="ExternalInput")
    woL = nc.dram_tensor("woL", (HEAD_DIM, HEADS, DIM), F8, kind="ExternalInput")
    f1H = nc.dram_tensor("f1H", (KT_D, 128, FF), F8, kind="ExternalInput")
    f1L = nc.dram_tensor("f1L", (KT_D, 128, FF), F8, kind="ExternalInput")
    f2H = nc.dram_tensor("f2H", (KT_F, 128, DIM), F8, kind="ExternalInput")
    f2L = nc.dram_tensor("f2L", (KT_F, 128, DIM), F8, kind="ExternalInput")
    pp = nc.dram_tensor("pp", (128, PP_COLS), F32, kind="ExternalInput")
    # bf16 row vectors (already *SW): vb2, ob, f2b, qb
    rowv = nc.dram_tensor("rowv", (4, DIM), BF, kind="ExternalInput")
    outT = nc.dram_tensor("outT", (KT_D, 128, TQ), F32, kind="ExternalOutput")

    with tile.TileContext(nc) as tc:
        from contextlib import ExitStack
        with ExitStack() as stk:
            pw = stk.enter_context(tc.tile_pool(name="pw", bufs=1))
            pact = stk.enter_context(tc.tile_pool(name="pact", bufs=1))
            prow = stk.enter_context(tc.tile_pool(name="prow", bufs=1))
            ptmp = stk.enter_context(tc.tile_pool(name="ptmp", bufs=2))
            psum = stk.enter_context(
                tc.tile_pool(name="psum", bufs=8, space="PSUM"))

            ones_sq = pw.tile([128, 128], BF, tag="ones")
            nc.vector.memset(ones_sq, 1.0)
            ones128 = ones_sq[:, 0:1]
            ones1 = ones_sq[0:1, :]
            onesrow = pw.tile([1, 512], BF, tag="onesrow")
            nc.vector.memset(onesrow, 1.0)
            pp_sb = pw.tile([128, PP_COLS], F32, tag="pp")
            nc.sync.dma_start(out=pp_sb, in_=pp.ap())
            vb_sb = pw.tile([1, DIM], BF, tag="vb")
            nc.sync.dma_start(out=vb_sb, in_=rowv.ap()[0:1])
            ob_sb = pw.tile([1, DIM], BF, tag="ob")
            nc.sync.dma_start(out=ob_sb, in_=rowv.ap()[1:2])
            f2b_sb = pw.tile([1, DIM], BF, tag="f2b")
            nc.sync.dma_start(out=f2b_sb, in_=rowv.ap()[2:3])
            qb_sb = pw.tile([1, DIM], BF, tag="qb")
            nc.sync.dma_start(out=qb_sb, in_=rowv.ap()[3:4])

            def ppc(col, n=1, rows=128):
                return pp_sb[:rows, col:col + n]

            def load3(pool, dram, shape, name):
                t = pool.tile(list(shape), dram.dtype, tag=name)
                for k in range(shape[1]):
                    nc.sync.dma_start(out=t[:, k, :], in_=dram.ap()[k])
                return t

            # ---------- LN helpers (g/b folded into weights host-side) ----
            def ln_stats_chunk(x_sb, n_kt, c0, cw, a_dst, m_dst):
                """a_dst <- 1/std, m_dst <- mean ([1, cw] APs) for x cols
                [c0, c0+cw)."""
                ps_s = psum.tile([128, 512], F32, tag="ps", name=f"ps_s{c0}_{x_sb.tensor.name}")
                ps_q = psum.tile([128, 512], F32, tag="ps", name=f"ps_q{c0}_{x_sb.tensor.name}")
                for kt in range(n_kt):
                    xs = x_sb[:, kt, c0:c0 + cw]
                    sq = ptmp.tile([128, 512], BF, tag="sq")
                    nc.gpsimd.tensor_tensor(out=sq[:, :cw], in0=xs, in1=xs,
                                            op=OP.mult)
                    nc.tensor.matmul(ps_s[:1, :cw], ones128, xs,
                                     start=(kt == 0), stop=(kt == n_kt - 1))
                    nc.tensor.matmul(ps_q[:1, :cw], ones128, sq[:, :cw],
                                     start=(kt == 0), stop=(kt == n_kt - 1))
                nc.scalar.activation(out=m_dst, in_=ps_s[:1, :cw], func=AF.Copy,
                                     scale=1.0 / DIM)
                c = ptmp.tile([1, 512], BF, tag="cvar")
                nc.vector.tensor_tensor(out=c[:, :cw], in0=m_dst, in1=m_dst,
                                        op=OP.mult)
                nc.vector.scalar_tensor_tensor(
                    out=c[:, :cw], in0=ps_q[:1, :cw], scalar=1.0 / DIM,
                    in1=c[:, :cw], op0=OP.mult, op1=OP.subtract)
                nc.scalar.activation(out=c[:, :cw], in_=c[:, :cw], func=AF.Ln,
                                     bias=pp_sb[:1, PP_EPS:PP_EPS + 1])
                nc.scalar.activation(out=a_dst, in_=c[:, :cw],
                                     func=AF.Exp, scale=-0.5)

            def ln_norm_chunk(x_view, out_view, n_kt, cw, a_src, m_src,
                              name=""):
                """out = (x - mean) * inv_std, fp8 out. a/m are [1,cw] APs."""
                ps_m = psum.tile([128, 512], F32, tag="ps", name=f"ps_m{name}")
                ps_i = psum.tile([128, 512], F32, tag="ps", name=f"ps_i{name}")
                nc.tensor.matmul(ps_m[:, :cw], ones1, m_src)
                nc.tensor.matmul(ps_i[:, :cw], ones1, a_src)
                for kt in range(n_kt):
                    t = ptmp.tile([128, 512], BF, tag="normt")
                    nc.vector.tensor_tensor(out=t[:, :cw],
                                            in0=x_view[:, kt, :cw],
                                            in1=ps_m[:, :cw], op=OP.subtract)
                    nc.vector.tensor_tensor(out=out_view[:, kt, :cw],
                                            in0=t[:, :cw],
                                            in1=ps_i[:, :cw], op=OP.mult)

            # ---------- persistent activations ----------
            llm_sb = pact.tile([128, KT_D, TQ], BF, tag="llm")
            k_sb = pact.tile([HEAD_DIM, HEADS, TK], F8, tag="k")
            v_sb = pact.tile([LC, BPC, DIM], F8, tag="v")

            a_2 = prow.tile([1, TQ], BF, tag="a_2")
            m_2 = prow.tile([1, TQ], BF, tag="m_2")
            prowb = stk.enter_context(tc.tile_pool(name="prowb", bufs=2))

            # ================= weights (persistent through PM) ============
            pqw = stk.enter_context(tc.tile_pool(name="pqw", bufs=1))
            pwproj = tc.alloc_tile_pool(name="pwproj", bufs=1)
            pemb = tc.alloc_tile_pool(name="pemb", bufs=2)
            wph_sb = load3(pwproj, wpH, (128, KT_L, DIM), "wph")
            wpl_sb = load3(pwproj, wpL, (128, KT_L, DIM), "wpl")
            wqh_sb = load3(pqw, wqH, (128, KT_D, DIM), "wqh")
            wql_sb = load3(pqw, wqL, (128, KT_D, DIM), "wql")
            woh_sb = pqw.tile([HEAD_DIM, HEADS, DIM], F8, tag="woh")
            wol_sb = pqw.tile([HEAD_DIM, HEADS, DIM], F8, tag="wol")
            for h in range(HEADS):
                nc.sync.dma_start(out=woh_sb[:, h, :], in_=woH.ap()[:, h, :])
                nc.sync.dma_start(out=wol_sb[:, h, :], in_=woL.ap()[:, h, :])

            # ====== clip path: LN_q + k + v =====
            with tc.tile_pool(name="pclip", bufs=1) as pclip, \
                 tc.tile_pool(name="pkvw", bufs=1) as pkvw:
                clip_sb = load3(pclip, clipT, (128, KT_D, TK), "clip")
                clipn_sb = pclip.tile([128, KT_D, TK], F8, tag="clipn")
                a_c = prow.tile([1, TK], BF, tag="a_c")
                m_c = prow.tile([1, TK], BF, tag="m_c")
                for ci in range(2):
                    c0 = ci * 308
                    ln_stats_chunk(clip_sb, KT_D, c0, 308,
                                   a_c[:, c0:c0 + 308], m_c[:, c0:c0 + 308])
                for ci in range(2):
                    c0 = ci * 308
                    ln_norm_chunk(clip_sb[:, :, c0:c0 + 308],
                                  clipn_sb[:, :, c0:c0 + 308], KT_D, 308,
                                  a_c[:, c0:c0 + 308], m_c[:, c0:c0 + 308],
                                  name=f"cl{ci}")

                wkh_sb = load3(pkvw, wkH, (128, KT_D, DIM), "wkh")
                wkl_sb = load3(pkvw, wkL, (128, KT_D, DIM), "wkl")
                wvh_sb = load3(pkvw, wvH, (128, KT_D, DIM), "wvh")
                wvl_sb = load3(pkvw, wvL, (128, KT_D, DIM), "wvl")

                # k.T head-major [96, h, 616]
                for h in range(HEADS):
                    for ci in range(2):
                        c0 = ci * 308
                        ps = psum.tile([128, 512], F32, tag="ps",
                                       name=f"ps_k{h}_{ci}")
                        idx = 0
                        for wt in (wkh_sb, wkl_sb):
                            for t in range(KT_D // 2):
                                nc.tensor.matmul(
                                    ps[:HEAD_DIM, :308],
                                    wt[:, 2 * t:2 * t + 2, h * 96:(h + 1) * 96],
                                    clipn_sb[:, 2 * t:2 * t + 2, c0:c0 + 308],
                                    start=(idx == 0), stop=(idx == 5),
                                    perf_mode=DR)
                                idx += 1
                        nc.scalar.activation(
                            out=k_sb[:, h, c0:c0 + 308],
                            in_=ps[:HEAD_DIM, :308], func=AF.Identity,
                            scale=1.0 / SW, bias=ppc(PP_KB + h, rows=96))

                # v token-major [77, b, 768]
                for b in range(BPC):
                    for ci in range(2):
                        c0 = ci * 384
                        ps = psum.tile([128, 512], F32, tag="ps",
                                       name=f"ps_v{b}_{ci}")
                        idx = 0
                        for wt in (wvh_sb, wvl_sb):
                            for t in range(KT_D):
                                nc.tensor.matmul(
                                    ps[:LC, :384],
                                    clipn_sb[:, t, b * LC:(b + 1) * LC],
                                    wt[:, t, c0:c0 + 384],
                                    start=(idx == 0), stop=False)
                                idx += 1
                        nc.tensor.matmul(ps[:LC, :384], ones1[:, :LC],
                                         vb_sb[:, c0:c0 + 384],
                                         start=False, stop=True)
                        nc.scalar.activation(
                            out=v_sb[:, b, c0:c0 + 384], in_=ps[:LC, :384],
                            func=AF.Copy, scale=1.0 / SW)

            # f1 weights on the right heap side: DMA overlaps attention;
            # f2 + out pools open on the left after proj/emb pools close.
            pf1 = stk.enter_context(tc.tile_pool(name="pf1", bufs=1,
                                                 side="right"))
            f1h_sb = load3(pf1, f1H, (128, KT_D, FF), "f1h")
            f1l_sb = load3(pf1, f1L, (128, KT_D, FF), "f1l")
            pffn = stk.enter_context(tc.tile_pool(name="pffn", bufs=2,
                                                  side="right"))
            pfc = stk.enter_context(tc.tile_pool(name="pfc", bufs=1,
                                                 side="right"))
            pmid = stk.enter_context(tc.tile_pool(name="pmid", bufs=2,
                                                  side="right"))
            patn = stk.enter_context(tc.tile_pool(name="patn", bufs=8,
                                                  side="right"))

            # ---------- per-batch emitters ----------
            embs = {}

            def emit_emb_dma(b):
                c0 = b * LL
                ehi = pemb.tile([128, KT_L, LL], F8, tag="ehi", name=f"ehi{b}")
                elo = pemb.tile([128, KT_L, LL], F8, tag="elo", name=f"elo{b}")
                for kt in range(KT_L):
                    nc.sync.dma_start(out=ehi[:, kt, :],
                                      in_=embH.ap()[kt, :, c0:c0 + LL])
                    nc.sync.dma_start(out=elo[:, kt, :],
                                      in_=embL.ap()[kt, :, c0:c0 + LL])
                embs[b] = (ehi, elo)

            def emit_proj_mt(b, mt):
                """One proj output tile [128, 256] for batch b."""
                ehi, elo = embs[b]
                c0 = b * LL
                ps = psum.tile([128, 512], F32, tag="ps", name=f"ps_p{b}_{mt}")
                idx = 0
                for et, wt in ((ehi, wph_sb), (ehi, wpl_sb), (elo, wph_sb)):
                    for t in range(KT_L // 2):
                        nc.tensor.matmul(
                            ps[:, :LL],
                            wt[:, 2 * t:2 * t + 2, mt * 128:(mt + 1) * 128],
                            et[:, 2 * t:2 * t + 2, :],
                            start=(idx == 0), stop=(idx == 23), perf_mode=DR)
                        idx += 1
                nc.scalar.activation(
                    out=llm_sb[:, mt, c0:c0 + LL], in_=ps[:, :LL],
                    func=AF.Identity, scale=1.0 / (SW * SE),
                    bias=ppc(PP_PROJB + mt))
                if mt == KT_D - 1:
                    embs.pop(b)

            lnns = {}
            kvrows = {}

            def emit_kv_stats(b):
                a_t = prowb.tile([1, LL], BF, tag="akv", name=f"akv{b}")
                m_t = prowb.tile([1, LL], BF, tag="mkv", name=f"mkv{b}")
                ln_stats_chunk(llm_sb, KT_D, b * LL, LL, a_t[0:1, :],
                               m_t[0:1, :])
                kvrows[b] = (a_t, m_t)

            def emit_lnn(b):
                t = pmid.tile([128, KT_D, LL], F8, tag="lnn", name=f"lnn{b}")
                cc = b * LL
                a_t, m_t = kvrows.pop(b)
                ln_norm_chunk(llm_sb[:, :, cc:cc + LL], t, KT_D, LL,
                              a_t[0:1, :], m_t[0:1, :], name=f"kv{b}")
                lnns[b] = t

            def emit_q(b):
                lnn = lnns.pop(b)
                t = pmid.tile([HEAD_DIM, HEADS, LL], F8, tag="q_c",
                              name=f"q_c{b}")
                for p in range(HEADS // 2):
                    ps = psum.tile([128, 512], F32, tag="ps",
                                   name=f"ps_qp_{b}_{p}")
                    for i, h in enumerate((2 * p, 2 * p + 1)):
                        co = i * LL
                        idx = 0
                        for wt in (wqh_sb, wql_sb):
                            for tt in range(KT_D // 2):
                                nc.tensor.matmul(
                                    ps[:HEAD_DIM, co:co + LL],
                                    wt[:, 2 * tt:2 * tt + 2,
                                       h * 96:(h + 1) * 96],
                                    lnn[:, 2 * tt:2 * tt + 2, :],
                                    start=(idx == 0 and i == 0), stop=False,
                                    perf_mode=DR)
                                idx += 1
                        # + qb (already *SW) broadcast over tokens
                        nc.tensor.matmul(
                            ps[:HEAD_DIM, co:co + LL],
                            qb_sb[:, h * 96:(h + 1) * 96], onesrow[:, :LL],
                            start=False, stop=(i == 1))
                    nc.scalar.activation(
                        out=t[:, 2 * p:2 * p + 2, :], in_=ps[:HEAD_DIM, :],
                        func=AF.Copy, scale=1.0 / SW)
                return t

            def emit_o(b, ao_c):
                c0 = b * LL
                for mt in range(KT_D):
                    ps = psum.tile([128, 512], F32, tag="ps",
                                   name=f"ps_o{b}_{mt}")
                    idx = 0
                    for wt in (woh_sb, wol_sb):
                        for hh in range(HEADS // 2):
                            nc.tensor.matmul(
                                ps[:, :LL],
                                wt[:, 2 * hh:2 * hh + 2,
                                   mt * 128:(mt + 1) * 128],
                                ao_c[:, 2 * hh:2 * hh + 2, :],
                                start=(idx == 0), stop=False, perf_mode=DR)
                            idx += 1
                    nc.tensor.matmul(ps[:, :LL],
                                     ob_sb[:, mt * 128:(mt + 1) * 128],
                                     onesrow[:, :LL], start=False, stop=True)
                    nc.vector.scalar_tensor_tensor(
                        out=llm_sb[:, mt, c0:c0 + LL], in0=ps[:, :LL],
                        scalar=1.0 / SW, in1=llm_sb[:, mt, c0:c0 + LL],
                        op0=OP.mult, op1=OP.add)

            # ---------- stage PM: proj + attention interleaved ----------
            emit_emb_dma(0)
            emit_emb_dma(1)
            for mt in range(KT_D):
                emit_proj_mt(0, mt)
            emit_kv_stats(0)
            emit_lnn(0)

            h_cs = {}

            def emit_h(cc):
                t = pffn.tile([128, KT_D, 512], F8, tag="h_c", name=f"h_c{cc}")
                c0 = cc * 512
                ln_norm_chunk(llm_sb[:, :, c0:c0 + 512],
                              t, KT_D, 512, a_2[:, c0:c0 + 512],
                              m_2[:, c0:c0 + 512], name=f"n2{cc}")
                h_cs[cc] = t

            f_cs = {}

            def emit_f1_mt(cc, mt):
                if cc not in f_cs:
                    f_cs[cc] = pfc.tile([128, KT_F, 512], F8, tag="f_c",
                                        name=f"f_c{cc}")
                f_c = f_cs[cc]
                h_c = h_cs[cc]
                ps = psum.tile([128, 512], F32, tag="ps", name=f"ps_f1_{cc}_{mt}")
                idx = 0
                for wt in (f1h_sb, f1l_sb):
                    for t in range(KT_D // 2):
                        nc.tensor.matmul(
                            ps, wt[:, 2 * t:2 * t + 2, mt * 128:(mt + 1) * 128],
                            h_c[:, 2 * t:2 * t + 2, :],
                            start=(idx == 0), stop=(idx == 5), perf_mode=DR)
                        idx += 1
                nc.scalar.activation(
                    out=f_c[:, mt, :], in_=ps, func=AF.Gelu_apprx_sigmoid,
                    scale=1.0 / SW, bias=ppc(PP_F1B + mt))

            for b in range(BPC):
                if b + 2 < BPC:
                    emit_emb_dma(b + 2)
                q_c = emit_q(b)

                # filler PE work while softmax runs on ACT/DVE
                filler = []
                if b + 1 < BPC:
                    filler = [lambda mt=mt: emit_proj_mt(b + 1, mt)
                              for mt in range(KT_D)]
                else:
                    # last batch: start FFN chunk 0's f1 as filler
                    emit_h(0)
                    filler = [lambda mt=mt: emit_f1_mt(0, mt)
                              for mt in range(6)]
                fi = iter(filler)

                def fill(n=1):
                    for _ in range(n):
                        f = next(fi, None)
                        if f is not None:
                            f()

                # attention pairs of heads: the pair shares PSUM banks
                # (head 2p in cols [0,256), head 2p+1 in [256,512)) so each
                # ACT/DVE op covers both heads in one instruction.
                ps1, ps2, psv, ex, inv = {}, {}, {}, {}, {}

                def sc(p):
                    ps1[p] = psum.tile([128, 512], F32, tag="ps",
                                       name=f"ps1_{b}_{p}")
                    for i, h in enumerate((2 * p, 2 * p + 1)):
                        nc.tensor.matmul(ps1[p][:LC, i * LL:(i + 1) * LL],
                                         k_sb[:, h, b * LC:(b + 1) * LC],
                                         q_c[:, h, :],
                                         start=(i == 0), stop=(i == 1))
                    ex[p] = patn.tile([LC, 2 * LL], F8, tag="ex",
                                      name=f"ex_{b}_{p}")
                    nc.scalar.activation(out=ex[p], in_=ps1[p][:LC, :],
                                         func=AF.Exp, scale=SCALE)

                def rs(p):
                    ps2[p] = psum.tile([128, 512], F32, tag="ps",
                                       name=f"ps2_{b}_{p}")
                    for i in range(2):
                        nc.tensor.matmul(ps2[p][:1, i * LL:(i + 1) * LL],
                                         ones128[:LC, :],
                                         ex[p][:, i * LL:(i + 1) * LL],
                                         start=(i == 0), stop=(i == 1))
                    inv[p] = patn.tile([1, 2 * LL], BF, tag="inv",
                                       name=f"inv_{b}_{p}")
                    with nc.allow_low_precision("softmax 1/sum bf16"):
                        nc.vector.reciprocal(out=inv[p], in_=ps2[p][:1, :])

                def bc(p):
                    for i in range(2):
                        nc.tensor.matmul(ps2[p][:LC, i * LL:(i + 1) * LL],
                                         ones1[:, :LC],
                                         inv[p][:, i * LL:(i + 1) * LL],
                                         start=(i == 0), stop=(i == 1))
                    nc.vector.tensor_tensor(out=ex[p], in0=ex[p],
                                            in1=ps2[p][:LC, :], op=OP.mult)

                ao_c = pmid.tile([HEAD_DIM, HEADS, LL], F8, tag="ao_c",
                                 name=f"ao_c{b}")

                def av(p):
                    psv[p] = psum.tile([128, 512], F32, tag="ps",
                                       name=f"psv_{b}_{p}")
                    for i, h in enumerate((2 * p, 2 * p + 1)):
                        nc.tensor.matmul(
                            psv[p][:HEAD_DIM, i * LL:(i + 1) * LL],
                            v_sb[:, b, h * 96:(h + 1) * 96],
                            ex[p][:, i * LL:(i + 1) * LL],
                            start=(i == 0), stop=(i == 1))
                    nc.scalar.activation(out=ao_c[:, 2 * p:2 * p + 2, :],
                                         in_=psv[p][:HEAD_DIM, :],
                                         func=AF.Copy)

                sc(0); fill(); rs(0); sc(1); bc(0); fill(); av(0)
                rs(1); sc(2); bc(1); fill(); av(1)
                rs(2); sc(3); bc(2); fill(); av(2)
                rs(3); fill(); bc(3); av(3); fill(2)

                if b + 1 < BPC:
                    emit_kv_stats(b + 1)
                    emit_lnn(b + 1)
                emit_o(b, ao_c)
                c0 = b * LL
                ln_stats_chunk(llm_sb, KT_D, c0, LL, a_2[:, c0:c0 + LL],
                               m_2[:, c0:c0 + LL])

            # ---------- stage F: FFN ----------
            pemb.release()
            pwproj.release()
            pf2 = stk.enter_context(tc.tile_pool(name="pf2", bufs=1))
            f2h_sb = load3(pf2, f2H, (128, KT_F, DIM), "f2h")
            f2l_sb = load3(pf2, f2L, (128, KT_F, DIM), "f2l")
            pout = stk.enter_context(tc.tile_pool(name="pout", bufs=2))
            NCH = 512
            NFC = TQ // NCH
            for ci in range(NFC):
                for mt in range(6 if ci == 0 else 0, KT_F):
                    emit_f1_mt(ci, mt)
                if ci + 1 < NFC:
                    emit_h(ci + 1)
                f_c = f_cs.pop(ci)
                h_cs.pop(ci)
                c0 = ci * NCH
                for mt in range(KT_D):
                    ps = psum.tile([128, 512], F32, tag="ps",
                                   name=f"ps_f2_{ci}_{mt}")
                    idx = 0
                    for wt in (f2h_sb, f2l_sb):
                        for t in range(KT_F // 2):
                            nc.tensor.matmul(
                                ps,
                                wt[:, 2 * t:2 * t + 2, mt * 128:(mt + 1) * 128],
                                f_c[:, 2 * t:2 * t + 2, :],
                                start=(idx == 0), stop=False, perf_mode=DR)
                            idx += 1
                    nc.tensor.matmul(ps, f2b_sb[:, mt * 128:(mt + 1) * 128],
                                     onesrow[:, :NCH], start=False, stop=True)
                    o_c = pout.tile([128, NCH], F32, tag="o_c")
                    nc.vector.scalar_tensor_tensor(
                        out=o_c, in0=ps, scalar=1.0 / SW,
                        in1=llm_sb[:, mt, c0:c0 + NCH],
                        op0=OP.mult, op1=OP.add)
                    nc.sync.dma_start(out=outT.ap()[mt, :, c0:c0 + NCH],
                                      in_=o_c)

    nc.compile()
    return nc


_NC_CACHE = {}


def _get_nc():
    if "nc" not in _NC_CACHE:
        _NC_CACHE["nc"] = build_nc()
    return _NC_CACHE["nc"]


def _hilo(w):
    """Split f32 array (already scaled) into fp8 hi/lo."""
    hi = w.astype(F8NP)
    lo = (w - hi.astype(np.float32)).astype(F8NP)
    return hi, lo


def _prep_in_maps(inputs):
    f32 = np.float32
    g = {k: np.asarray(v, f32) for k, v in inputs.items()}

    # fold LN gains/biases into consuming weights
    kw = g["k_w"] * g["qn_g"][None, :]
    kb = g["k_b"] + g["k_w"] @ g["qn_b"]
    vw = g["v_w"] * g["qn_g"][None, :]
    vb = g["v_b"] + g["v_w"] @ g["qn_b"]
    qw = g["q_w"] * g["kvn_g"][None, :]
    qb = g["q_b"] + g["q_w"] @ g["kvn_b"]
    f1w = g["f1_w"] * g["n_g"][None, :]
    f1b = g["f1_b"] + g["f1_w"] @ g["n_b"]

    w = {}
    def put_hl(name, arr):
        hi, lo = _hilo(arr * SW)
        w[name + "H"] = np.ascontiguousarray(hi)
        w[name + "L"] = np.ascontiguousarray(lo)

    put_hl("wp", g["llm_proj_w"].T.reshape(KT_L, 128, DIM))
    put_hl("wq", qw.T.reshape(KT_D, 128, DIM))
    put_hl("wk", kw.T.reshape(KT_D, 128, DIM))
    put_hl("wv", vw.T.reshape(KT_D, 128, DIM))
    put_hl("wo", np.ascontiguousarray(
        g["o_w"].T.reshape(HEADS, HEAD_DIM, DIM).transpose(1, 0, 2)))
    put_hl("f1", f1w.T.reshape(KT_D, 128, FF))
    put_hl("f2", g["f2_w"].T.reshape(KT_F, 128, DIM))

    rowv = np.zeros((4, DIM), f32)
    rowv[0] = vb * SW
    rowv[1] = g["o_b"] * SW
    rowv[2] = g["f2_b"] * SW
    rowv[3] = qb * SW
    w["rowv"] = rowv.astype(BF16)

    ppa = np.zeros((128, PP_COLS), dtype=f32)

    def put(col, vec, n):
        ppa[:, col:col + n] = np.asarray(vec, dtype=f32).reshape(n, 128).T

    put(PP_PROJB, g["llm_proj_b"], KT_D)
    put(PP_F1B, f1b, KT_F)
    ppa[:HEAD_DIM, PP_QB:PP_QB + HEADS] = qb.reshape(HEADS, HEAD_DIM).T
    ppa[:HEAD_DIM, PP_KB:PP_KB + HEADS] = kb.reshape(HEADS, HEAD_DIM).T
    ppa[:, PP_EPS] = EPS
    w["pp"] = ppa

    clip = g["clip_embed"]
    llm = g["llm_embed"]
    in_maps = []
    for c in range(NCORES):
        cs = slice(c * BPC, (c + 1) * BPC)
        m = dict(w)
        embT = llm[cs].reshape(TQ, LLAMA_DIM).T.reshape(KT_L, 128, TQ) * SE
        ehi, elo = _hilo(embT)
        m["embH"] = np.ascontiguousarray(ehi)
        m["embL"] = np.ascontiguousarray(elo)
        m["clipT"] = np.ascontiguousarray(
            clip[cs].reshape(TK, DIM).T.reshape(KT_D, 128, TK)).astype(BF16)
        in_maps.append(m)
    return in_maps


def run(inputs, trace=False):
    nc = _get_nc()
    in_maps = _prep_in_maps(inputs)
    res = bass_utils.run_bass_kernel_spmd(
        nc, in_maps, core_ids=list(range(NCORES)), trace=trace)
    clip = np.asarray(inputs["clip_embed"], dtype=np.float32)
    llm3 = np.empty((B, LL, DIM), dtype=np.float32)
    for c in range(NCORES):
        yT = res.results[c]["outT"].reshape(DIM, TQ)
        llm3[c * BPC:(c + 1) * BPC] = yT.T.reshape(BPC, LL, DIM)
    out = np.concatenate([clip, llm3], axis=1)
    return out, res


def kernel(**inputs):
    out, _ = run(inputs, trace=False)
    return out


# revision 13
# speedup vs baseline: 1.1572x; 1.0196x over previous
"""CrossFusion block on 8 TRN2 NeuronCores.

Data-parallel over batch: 64 batches -> 8 cores x 8 batches.
Feature-major activations (feature dim on SBUF partitions); all layout
transposition host-side.

v3: all linear layers run fp8(e4m3) matmuls with hi/lo split-precision
compensation; pairs of 128-deep contraction tiles go through DoubleRow
matmuls (2 k-tiles per pass). The proj layer additionally compensates
the activation side (llm_embed split hi/lo host-side, 3-product chain)
because its error feeds the residual stream directly. LayerNorm
gains/biases are folded into the consuming weights host-side, so
on-device LN is xn = (x - mean) * inv_std only.

Activation-table discipline: stage P (proj + both input LNs + k/v) uses
only Sqrt/Copy/Identity; stage M (attention) only Exp; stage F (FFN)
Sqrt tails then Gelu -> 3 table loads total. LN2 stat matmuls run
inside stage M (squares on Pool, sums on PE, mean/var via DVE from
PSUM) but their Sqrt tails are deferred to stage F via a variance row.

Attention packs head pairs into shared PSUM banks (head 2p in columns
[0,256), head 2p+1 in [256,512)) so each softmax ACT/DVE op covers two
heads in one instruction. q-proj for batch b+1 is emitted between the
rowsum and broadcast matmuls of batch b to keep the PE busy during
softmax ACT/DVE latency.
"""

import sys

sys.path.insert(0, "/opt/trn_rl_repo")

import numpy as np
import ml_dtypes

import concourse.bass as bass
import concourse.tile as tile
from concourse import bacc, mybir
from concourse import bass_utils

BF16 = ml_dtypes.bfloat16
F8NP = ml_dtypes.float8_e4m3

B, LC, LL, LLAMA_DIM, DIM, HEADS = 64, 77, 256, 2048, 768, 8
HEAD_DIM = DIM // HEADS          # 96
SCALE = HEAD_DIM ** -0.5
FF = 4 * DIM                     # 3072
NCORES = 8
BPC = B // NCORES                # batches per core = 8
TQ = BPC * LL                    # llm tokens per core = 2048
TK = BPC * LC                    # clip tokens per core = 616
KT_D = DIM // 128                # 6
KT_L = LLAMA_DIM // 128          # 16
KT_F = FF // 128                 # 24
EPS = 1e-5
SW = 256.0                       # fp8 weight scale
SE = 16.0                        # fp8 llm_embed scale

F32 = mybir.dt.float32
BF = mybir.dt.bfloat16
F8 = mybir.dt.float8e4
AF = mybir.ActivationFunctionType
OP = mybir.AluOpType
DR = mybir.MatmulPerfMode.DoubleRow

# packed param tile column offsets (all f32, [128, PP_COLS])
PP_PROJB = 0          # 6
PP_F1B = 6            # 24
PP_KB = 30            # 8 cols (rows 0..95)
PP_EPS = 38
PP_COLS = 39


def build_nc():
    nc = bacc.Bacc("TRN2", target_bir_lowering=False, debug=False)

    embH = nc.dram_tensor("embH", (KT_L, 128, TQ), F8, kind="ExternalInput")
    embL = nc.dram_tensor("embL", (KT_L, 128, TQ), F8, kind="ExternalInput")
    clipT = nc.dram_tensor("clipT", (KT_D, 128, TK), BF, kind="ExternalInput")
    wpH = nc.dram_tensor("wpH", (KT_L, 128, DIM), F8, kind="ExternalInput")
    wpL = nc.dram_tensor("wpL", (KT_L, 128, DIM), F8, kind="ExternalInput")
    wqH = nc.dram_tensor("wqH", (KT_D, 128, DIM), F8, kind="ExternalInput")
    wqL = nc.dram_tensor("wqL", (KT_D, 128, DIM), F8, kind="ExternalInput")
    wkH = nc.dram_tensor("wkH", (KT_D, 128, DIM), F8, kind="ExternalInput")
    wkL = nc.dram_tensor("wkL", (KT_D, 128, DIM), F8, kind="ExternalInput")
    wvH = nc.dram_tensor("wvH", (KT_D, 128, DIM), F8, kind="ExternalInput")
    wvL = nc.dram_tensor("wvL", (KT_D, 128, DIM), F8, kind="ExternalInput")
    woH = nc.dram_tensor("woH", (HEAD_DIM, HEADS, DIM), F8, kind="ExternalInput")
    woL = nc.dram_tensor("woL", (HEAD_DIM, HEADS, DIM), F8, kind="ExternalInput")
    f1H = nc.dram_tensor("f1H", (KT_D, 128, FF), F8, kind="ExternalInput")
    f1L = nc.dram_tensor("f1L", (KT_D, 128, FF), F8, kind="ExternalInput")
    f2H = nc.dram_tensor("f2H", (KT_F, 128, DIM), F8, kind="ExternalInput")
    f2L = nc.dram_tensor("f2L", (KT_F, 128, DIM), F8, kind="ExternalInput")
    pp = nc.dram_tensor("pp", (128, PP_COLS), F32, kind="ExternalInput")
    # bf16 row vectors (already *SW): vb2, ob, f2b, qb
    rowv = nc.dram_tensor("rowv", (4, DIM), BF, kind="ExternalInput")
    outT = nc.dram_tensor("outT", (KT_D, 128, TQ), F32, kind="ExternalOutput")

    with tile.TileContext(nc) as tc:
        from contextlib import ExitStack
        with ExitStack() as stk:
            pw = stk.enter_context(tc.tile_pool(name="pw", bufs=1))
            pact = stk.enter_context(tc.tile_pool(name="pact", bufs=1))
            prow = stk.enter_context(tc.tile_pool(name="prow", bufs=1))
            prowb = stk.enter_context(tc.tile_pool(name="prowb", bufs=2))
            ptmp = stk.enter_context(tc.tile_pool(name="ptmp", bufs=2))
            psum = stk.enter_context(
                tc.tile_pool(name="psum", bufs=8, space="PSUM"))

            ones_sq = pw.tile([128, 128], BF, tag="ones")
            nc.vector.memset(ones_sq, 1.0)
            ones128 = ones_sq[:, 0:1]
            ones1 = ones_sq[0:1, :]
            onesrow = pw.tile([1, 512], BF, tag="onesrow")
            nc.vector.memset(onesrow, 1.0)
            pp_sb = pw.tile([128, PP_COLS], F32, tag="pp")
            nc.sync.dma_start(out=pp_sb, in_=pp.ap())
            vb_sb = pw.tile([1, DIM], BF, tag="vb")
            nc.sync.dma_start(out=vb_sb, in_=rowv.ap()[0:1])
            ob_sb = pw.tile([1, DIM], BF, tag="ob")
            nc.sync.dma_start(out=ob_sb, in_=rowv.ap()[1:2])
            f2b_sb = pw.tile([1, DIM], BF, tag="f2b")
            nc.sync.dma_start(out=f2b_sb, in_=rowv.ap()[2:3])
            qb_sb = pw.tile([1, DIM], BF, tag="qb")
            nc.sync.dma_start(out=qb_sb, in_=rowv.ap()[3:4])

            def ppc(col, n=1, rows=128):
                return pp_sb[:rows, col:col + n]

            def load3(pool, dram, shape, name):
                t = pool.tile(list(shape), dram.dtype, tag=name)
                for k in range(shape[1]):
                    nc.sync.dma_start(out=t[:, k, :], in_=dram.ap()[k])
                return t

            # ---------- LN helpers (g/b folded into weights host-side) ----
            def stats_mm(x_sb, n_kt, c0, cw, m_dst, var_dst):
                """PE/Pool/DVE part of LN stats: mean + variance rows."""
                ps_s = psum.tile([128, 512], F32, tag="ps",
                                 name=f"ps_s{c0}_{x_sb.tensor.name}")
                ps_q = psum.tile([128, 512], F32, tag="ps",
                                 name=f"ps_q{c0}_{x_sb.tensor.name}")
                for kt in range(n_kt):
                    xs = x_sb[:, kt, c0:c0 + cw]
                    sq = ptmp.tile([128, 512], BF, tag="sq")
                    nc.gpsimd.tensor_tensor(out=sq[:, :cw], in0=xs, in1=xs,
                                            op=OP.mult)
                    nc.tensor.matmul(ps_s[:1, :cw], ones128, xs,
                                     start=(kt == 0), stop=(kt == n_kt - 1))
                    nc.tensor.matmul(ps_q[:1, :cw], ones128, sq[:, :cw],
                                     start=(kt == 0), stop=(kt == n_kt - 1))
                nc.vector.tensor_scalar(out=m_dst, in0=ps_s[:1, :cw],
                                        scalar1=1.0 / DIM, scalar2=None,
                                        op0=OP.mult)
                c = ptmp.tile([1, 512], BF, tag="cvar")
                nc.vector.tensor_tensor(out=c[:, :cw], in0=m_dst, in1=m_dst,
                                        op=OP.mult)
                nc.vector.scalar_tensor_tensor(
                    out=var_dst, in0=ps_q[:1, :cw], scalar=1.0 / DIM,
                    in1=c[:, :cw], op0=OP.mult, op1=OP.subtract)

            def stats_tail(var_src, a_dst):
                """ACT Sqrt + DVE reciprocal: a_dst <- 1/sqrt(var+eps)."""
                nc.scalar.activation(out=var_src, in_=var_src, func=AF.Sqrt,
                                     bias=pp_sb[:1, PP_EPS:PP_EPS + 1])
                with nc.allow_low_precision("ln inv_std bf16"):
                    nc.vector.reciprocal(out=a_dst, in_=var_src)

            def ln_stats(x_sb, n_kt, c0, cw, a_dst, m_dst):
                v = ptmp.tile([1, 512], BF, tag="vrow")
                stats_mm(x_sb, n_kt, c0, cw, m_dst, v[:, :cw])
                stats_tail(v[:, :cw], a_dst)

            def ln_norm_chunk(x_view, out_view, n_kt, cw, a_src, m_src,
                              name=""):
                """out = (x - mean) * inv_std, fp8 out. a/m are [1,cw] APs."""
                ps_m = psum.tile([128, 512], F32, tag="ps", name=f"ps_m{name}")
                ps_i = psum.tile([128, 512], F32, tag="ps", name=f"ps_i{name}")
                nc.tensor.matmul(ps_m[:, :cw], ones1, m_src)
                nc.tensor.matmul(ps_i[:, :cw], ones1, a_src)
                for kt in range(n_kt):
                    t = ptmp.tile([128, 512], BF, tag="normt")
                    nc.vector.tensor_tensor(out=t[:, :cw],
                                            in0=x_view[:, kt, :cw],
                                            in1=ps_m[:, :cw], op=OP.subtract)
                    nc.vector.tensor_tensor(out=out_view[:, kt, :cw],
                                            in0=t[:, :cw],
                                            in1=ps_i[:, :cw], op=OP.mult)

            # ---------- persistent activations ----------
            llm_sb = pact.tile([128, KT_D, TQ], BF, tag="llm")
            llmn_sb = pact.tile([128, KT_D, TQ], F8, tag="llmn")
            k_sb = pact.tile([HEAD_DIM, HEADS, TK], F8, tag="k")
            v_sb = pact.tile([LC, BPC, DIM], F8, tag="v")

            a_2 = prow.tile([1, TQ], BF, tag="a_2")
            m_2 = prow.tile([1, TQ], BF, tag="m_2")
            v_2 = prow.tile([1, TQ], BF, tag="v_2")

            # ================= weights ====================================
            pqw = stk.enter_context(tc.tile_pool(name="pqw", bufs=1))
            pwproj = tc.alloc_tile_pool(name="pwproj", bufs=1)
            pemb = tc.alloc_tile_pool(name="pemb", bufs=2)
            wph_sb = load3(pwproj, wpH, (128, KT_L, DIM), "wph")
            wpl_sb = load3(pwproj, wpL, (128, KT_L, DIM), "wpl")
            wqh_sb = load3(pqw, wqH, (128, KT_D, DIM), "wqh")
            wql_sb = load3(pqw, wqL, (128, KT_D, DIM), "wql")
            woh_sb = pqw.tile([HEAD_DIM, HEADS, DIM], F8, tag="woh")
            wol_sb = pqw.tile([HEAD_DIM, HEADS, DIM], F8, tag="wol")
            for h in range(HEADS):
                nc.sync.dma_start(out=woh_sb[:, h, :], in_=woH.ap()[:, h, :])
                nc.sync.dma_start(out=wol_sb[:, h, :], in_=woL.ap()[:, h, :])

            # ====== stage P part 1: clip path (LN_q + k + v) ==============
            with tc.tile_pool(name="pclip", bufs=1) as pclip, \
                 tc.tile_pool(name="pkvw", bufs=1) as pkvw:
                clip_sb = load3(pclip, clipT, (128, KT_D, TK), "clip")
                clipn_sb = pclip.tile([128, KT_D, TK], F8, tag="clipn")
                a_c = prow.tile([1, TK], BF, tag="a_c")
                m_c = prow.tile([1, TK], BF, tag="m_c")
                for ci in range(2):
                    c0 = ci * 308
                    ln_stats(clip_sb, KT_D, c0, 308,
                             a_c[:, c0:c0 + 308], m_c[:, c0:c0 + 308])
                for ci in range(2):
                    c0 = ci * 308
                    ln_norm_chunk(clip_sb[:, :, c0:c0 + 308],
                                  clipn_sb[:, :, c0:c0 + 308], KT_D, 308,
                                  a_c[:, c0:c0 + 308], m_c[:, c0:c0 + 308],
                                  name=f"cl{ci}")

                wkh_sb = load3(pkvw, wkH, (128, KT_D, DIM), "wkh")
                wkl_sb = load3(pkvw, wkL, (128, KT_D, DIM), "wkl")
                wvh_sb = load3(pkvw, wvH, (128, KT_D, DIM), "wvh")
                wvl_sb = load3(pkvw, wvL, (128, KT_D, DIM), "wvl")

                # k.T head-major [96, h, 616]
                for h in range(HEADS):
                    for ci in range(2):
                        c0 = ci * 308
                        ps = psum.tile([128, 512], F32, tag="ps",
                                       name=f"ps_k{h}_{ci}")
                        idx = 0
                        for wt in (wkh_sb, wkl_sb):
                            for t in range(KT_D // 2):
                                nc.tensor.matmul(
                                    ps[:HEAD_DIM, :308],
                                    wt[:, 2 * t:2 * t + 2, h * 96:(h + 1) * 96],
                                    clipn_sb[:, 2 * t:2 * t + 2, c0:c0 + 308],
                                    start=(idx == 0), stop=(idx == 5),
                                    perf_mode=DR)
                                idx += 1
                        nc.scalar.activation(
                            out=k_sb[:, h, c0:c0 + 308],
                            in_=ps[:HEAD_DIM, :308], func=AF.Identity,
                            scale=1.0 / SW, bias=ppc(PP_KB + h, rows=96))

                # v token-major [77, b, 768] (activation stationary: plain
                # fp8, M=77 violates dual-fp8 ldweights restrictions)
                for b in range(BPC):
                    for ci in range(2):
                        c0 = ci * 384
                        ps = psum.tile([128, 512], F32, tag="ps",
                                       name=f"ps_v{b}_{ci}")
                        idx = 0
                        for wt in (wvh_sb, wvl_sb):
                            for t in range(KT_D):
                                nc.tensor.matmul(
                                    ps[:LC, :384],
                                    clipn_sb[:, t, b * LC:(b + 1) * LC],
                                    wt[:, t, c0:c0 + 384],
                                    start=(idx == 0), stop=False)
                                idx += 1
                        nc.tensor.matmul(ps[:LC, :384], ones1[:, :LC],
                                         vb_sb[:, c0:c0 + 384],
                                         start=False, stop=True)
                        nc.scalar.activation(
                            out=v_sb[:, b, c0:c0 + 384], in_=ps[:LC, :384],
                            func=AF.Copy, scale=1.0 / SW)

            # f1 weights on the right heap side: DMA overlaps stage P/M
            pf1 = stk.enter_context(tc.tile_pool(name="pf1", bufs=1,
                                                 side="right"))
            f1h_sb = load3(pf1, f1H, (128, KT_D, FF), "f1h")
            f1l_sb = load3(pf1, f1L, (128, KT_D, FF), "f1l")
            pffn = stk.enter_context(tc.tile_pool(name="pffn", bufs=2,
                                                  side="right"))
            pfc = stk.enter_context(tc.tile_pool(name="pfc", bufs=1,
                                                 side="right"))
            pmid = stk.enter_context(tc.tile_pool(name="pmid", bufs=2,
                                                  side="right"))
            patn = stk.enter_context(tc.tile_pool(name="patn", bufs=4,
                                                  side="right"))

            # ====== stage P part 2: proj + LN_kv + lnn (all batches) ======
            embs = {}

            def emit_emb_dma(b):
                c0 = b * LL
                ehi = pemb.tile([128, KT_L, LL], F8, tag="ehi", name=f"ehi{b}")
                elo = pemb.tile([128, KT_L, LL], F8, tag="elo", name=f"elo{b}")
                for kt in range(KT_L):
                    nc.sync.dma_start(out=ehi[:, kt, :],
                                      in_=embH.ap()[kt, :, c0:c0 + LL])
                    nc.sync.dma_start(out=elo[:, kt, :],
                                      in_=embL.ap()[kt, :, c0:c0 + LL])
                embs[b] = (ehi, elo)

            def emit_proj_mt(b, mt):
                """One proj output tile [128, 256] for batch b."""
                ehi, elo = embs[b]
                c0 = b * LL
                ps = psum.tile([128, 512], F32, tag="ps", name=f"ps_p{b}_{mt}")
                idx = 0
                for et, wt in ((ehi, wph_sb), (ehi, wpl_sb), (elo, wph_sb)):
                    for t in range(KT_L // 2):
                        nc.tensor.matmul(
                            ps[:, :LL],
                            wt[:, 2 * t:2 * t + 2, mt * 128:(mt + 1) * 128],
                            et[:, 2 * t:2 * t + 2, :],
                            start=(idx == 0), stop=(idx == 23), perf_mode=DR)
                        idx += 1
                nc.scalar.activation(
                    out=llm_sb[:, mt, c0:c0 + LL], in_=ps[:, :LL],
                    func=AF.Identity, scale=1.0 / (SW * SE),
                    bias=ppc(PP_PROJB + mt))
                if mt == KT_D - 1:
                    embs.pop(b)

            emit_emb_dma(0)
            emit_emb_dma(1)
            for b in range(BPC):
                if b + 2 < BPC:
                    emit_emb_dma(b + 2)
                for mt in range(KT_D):
                    emit_proj_mt(b, mt)
                c0 = b * LL
                a_t = prowb.tile([1, LL], BF, tag="akv", name=f"akv{b}")
                m_t = prowb.tile([1, LL], BF, tag="mkv", name=f"mkv{b}")
                ln_stats(llm_sb, KT_D, c0, LL, a_t[0:1, :], m_t[0:1, :])
                ln_norm_chunk(llm_sb[:, :, c0:c0 + LL],
                              llmn_sb[:, :, c0:c0 + LL], KT_D, LL,
                              a_t[0:1, :], m_t[0:1, :], name=f"kv{b}")
            pemb.release()
            pwproj.release()

            # f2 weights: DMA overlaps stage M
            pf2 = stk.enter_context(tc.tile_pool(name="pf2", bufs=1))
            f2h_sb = load3(pf2, f2H, (128, KT_F, DIM), "f2h")
            f2l_sb = load3(pf2, f2L, (128, KT_F, DIM), "f2l")
            pout = stk.enter_context(tc.tile_pool(name="pout", bufs=2))

            # ====== stage M: attention (Exp only on ACT) ==================
            def emit_q_pair(b, p, t):
                ps = psum.tile([128, 512], F32, tag="ps",
                               name=f"ps_qp_{b}_{p}")
                for i, h in enumerate((2 * p, 2 * p + 1)):
                    co = i * LL
                    idx = 0
                    for wt in (wqh_sb, wql_sb):
                        for tt in range(KT_D // 2):
                            nc.tensor.matmul(
                                ps[:HEAD_DIM, co:co + LL],
                                wt[:, 2 * tt:2 * tt + 2,
                                   h * 96:(h + 1) * 96],
                                llmn_sb[:, 2 * tt:2 * tt + 2,
                                        b * LL:(b + 1) * LL],
                                start=(idx == 0 and i == 0), stop=False,
                                perf_mode=DR)
                            idx += 1
                    nc.tensor.matmul(
                        ps[:HEAD_DIM, co:co + LL],
                        qb_sb[:, h * 96:(h + 1) * 96], onesrow[:, :LL],
                        start=False, stop=(i == 1))
                nc.scalar.activation(
                    out=t[:, 2 * p:2 * p + 2, :], in_=ps[:HEAD_DIM, :],
                    func=AF.Copy, scale=1.0 / SW)

            def emit_o(b, ao_c):
                c0 = b * LL
                for mt in range(KT_D):
                    ps = psum.tile([128, 512], F32, tag="ps",
                                   name=f"ps_o{b}_{mt}")
                    idx = 0
                    for wt in (woh_sb, wol_sb):
                        for hh in range(HEADS // 2):
                            nc.tensor.matmul(
                                ps[:, :LL],
                                wt[:, 2 * hh:2 * hh + 2,
                                   mt * 128:(mt + 1) * 128],
                                ao_c[:, 2 * hh:2 * hh + 2, :],
                                start=(idx == 0), stop=False, perf_mode=DR)
                            idx += 1
                    nc.tensor.matmul(ps[:, :LL],
                                     ob_sb[:, mt * 128:(mt + 1) * 128],
                                     onesrow[:, :LL], start=False, stop=True)
                    nc.vector.scalar_tensor_tensor(
                        out=llm_sb[:, mt, c0:c0 + LL], in0=ps[:, :LL],
                        scalar=1.0 / SW, in1=llm_sb[:, mt, c0:c0 + LL],
                        op0=OP.mult, op1=OP.add)

            q_cs = {}

            def emit_q(b, pairs):
                if b not in q_cs:
                    q_cs[b] = pmid.tile([HEAD_DIM, HEADS, LL], F8, tag="q_c",
                                        name=f"q_c{b}")
                for p in pairs:
                    emit_q_pair(b, p, q_cs[b])

            emit_q(0, range(4))
            for b in range(BPC):
                q_c = q_cs.pop(b)
                ps1, ps2, psv, ex, inv = {}, {}, {}, {}, {}

                def sc(p):
                    ps1[p] = psum.tile([128, 512], F32, tag="ps",
                                       name=f"ps1_{b}_{p}")
                    for i, h in enumerate((2 * p, 2 * p + 1)):
                        nc.tensor.matmul(ps1[p][:LC, i * LL:(i + 1) * LL],
                                         k_sb[:, h, b * LC:(b + 1) * LC],
                                         q_c[:, h, :],
                                         start=(i == 0), stop=(i == 1))
                    ex[p] = patn.tile([LC, 2 * LL], F8, tag="ex",
                                      name=f"ex_{b}_{p}")
                    nc.scalar.activation(out=ex[p], in_=ps1[p][:LC, :],
                                         func=AF.Exp, scale=SCALE)

                def rs(p):
                    ps2[p] = psum.tile([128, 512], F32, tag="ps",
                                       name=f"ps2_{b}_{p}")
                    for i in range(2):
                        nc.tensor.matmul(ps2[p][:1, i * LL:(i + 1) * LL],
                                         ones128[:LC, :],
                                         ex[p][:, i * LL:(i + 1) * LL],
                                         start=(i == 0), stop=(i == 1))
                    inv[p] = patn.tile([1, 2 * LL], BF, tag="inv",
                                       name=f"inv_{b}_{p}")
                    with nc.allow_low_precision("softmax 1/sum bf16"):
                        nc.vector.reciprocal(out=inv[p], in_=ps2[p][:1, :])

                def bc(p):
                    for i in range(2):
                        nc.tensor.matmul(ps2[p][:LC, i * LL:(i + 1) * LL],
                                         ones1[:, :LC],
                                         inv[p][:, i * LL:(i + 1) * LL],
                                         start=(i == 0), stop=(i == 1))
                    nc.vector.tensor_tensor(out=ex[p], in0=ex[p],
                                            in1=ps2[p][:LC, :], op=OP.mult)

                ao_c = pmid.tile([HEAD_DIM, HEADS, LL], F8, tag="ao_c",
                                 name=f"ao_c{b}")

                def av(p):
                    psv[p] = psum.tile([128, 512], F32, tag="ps",
                                       name=f"psv_{b}_{p}")
                    for i, h in enumerate((2 * p, 2 * p + 1)):
                        nc.tensor.matmul(
                            psv[p][:HEAD_DIM, i * LL:(i + 1) * LL],
                            v_sb[:, b, h * 96:(h + 1) * 96],
                            ex[p][:, i * LL:(i + 1) * LL],
                            start=(i == 0), stop=(i == 1))
                    nc.scalar.activation(out=ao_c[:, 2 * p:2 * p + 2, :],
                                         in_=psv[p][:HEAD_DIM, :],
                                         func=AF.Copy)

                for p in range(4):
                    sc(p)
                if b + 1 < BPC:
                    emit_q(b + 1, (0, 1))
                for p in range(4):
                    rs(p)
                if b + 1 < BPC:
                    emit_q(b + 1, (2, 3))
                for p in range(4):
                    bc(p)
                for p in range(4):
                    av(p)
                emit_o(b, ao_c)
                # LN2 stat matmuls (no ACT table use; Sqrt tail deferred)
                c0 = b * LL
                stats_mm(llm_sb, KT_D, c0, LL, m_2[:, c0:c0 + LL],
                         v_2[:, c0:c0 + LL])

            # ====== stage F: LN2 tails + FFN ==============================
            for c in range(4):
                c0 = c * 512
                stats_tail(v_2[:, c0:c0 + 512], a_2[:, c0:c0 + 512])

            h_cs = {}

            def emit_h(cc):
                t = pffn.tile([128, KT_D, 512], F8, tag="h_c", name=f"h_c{cc}")
                c0 = cc * 512
                ln_norm_chunk(llm_sb[:, :, c0:c0 + 512],
                              t, KT_D, 512, a_2[:, c0:c0 + 512],
                              m_2[:, c0:c0 + 512], name=f"n2{cc}")
                h_cs[cc] = t

            f_cs = {}

            def emit_f1_mt(cc, mt):
                if cc not in f_cs:
                    f_cs[cc] = pfc.tile([128, KT_F, 512], F8, tag="f_c",
                                        name=f"f_c{cc}")
                f_c = f_cs[cc]
                h_c = h_cs[cc]
                ps = psum.tile([128, 512], F32, tag="ps",
                               name=f"ps_f1_{cc}_{mt}")
                idx = 0
                for wt in (f1h_sb, f1l_sb):
                    for t in range(KT_D // 2):
                        nc.tensor.matmul(
                            ps, wt[:, 2 * t:2 * t + 2, mt * 128:(mt + 1) * 128],
                            h_c[:, 2 * t:2 * t + 2, :],
                            start=(idx == 0), stop=(idx == 5), perf_mode=DR)
                        idx += 1
                nc.scalar.activation(
                    out=f_c[:, mt, :], in_=ps, func=AF.Gelu_apprx_sigmoid,
                    scale=1.0 / SW, bias=ppc(PP_F1B + mt))

            NCH = 512
            NFC = TQ // NCH
            emit_h(0)
            for ci in range(NFC):
                for mt in range(KT_F):
                    emit_f1_mt(ci, mt)
                if ci + 1 < NFC:
                    emit_h(ci + 1)
                f_c = f_cs.pop(ci)
                h_cs.pop(ci)
                c0 = ci * NCH
                for mt in range(KT_D):
                    ps = psum.tile([128, 512], F32, tag="ps",
                                   name=f"ps_f2_{ci}_{mt}")
                    idx = 0
                    for wt in (f2h_sb, f2l_sb):
                        for t in range(KT_F // 2):
                            nc.tensor.matmul(
                                ps,
                                wt[:, 2 * t:2 * t + 2, mt * 128:(mt + 1) * 128],
                                f_c[:, 2 * t:2 * t + 2, :],
                                start=(idx == 0), stop=False, perf_mode=DR)
                            idx += 1
                    nc.tensor.matmul(ps, f2b_sb[:, mt * 128:(mt + 1) * 128],
                                     onesrow[:, :NCH], start=False, stop=True)
                    o_c = pout.tile([128, NCH], F32, tag="o_c")
                    nc.vector.scalar_tensor_tensor(
                        out=o_c, in0=ps, scalar=1.0 / SW,
                        in1=llm_sb[:, mt, c0:c0 + NCH],
                        op0=OP.mult, op1=OP.add)
                    nc.sync.dma_start(out=outT.ap()[mt, :, c0:c0 + NCH],
                                      in_=o_c)

    nc.compile()
    return nc


_NC_CACHE = {}


def _get_nc():
    if "nc" not in _NC_CACHE:
        _NC_CACHE["nc"] = build_nc()
    return _NC_CACHE["nc"]


def _hilo(w):
    """Split f32 array (already scaled) into fp8 hi/lo."""
    hi = w.astype(F8NP)
    lo = (w - hi.astype(np.float32)).astype(F8NP)
    return hi, lo


def _prep_in_maps(inputs):
    f32 = np.float32
    g = {k: np.asarray(v, f32) for k, v in inputs.items()}

    # fold LN gains/biases into consuming weights
    kw = g["k_w"] * g["qn_g"][None, :]
    kb = g["k_b"] + g["k_w"] @ g["qn_b"]
    vw = g["v_w"] * g["qn_g"][None, :]
    vb = g["v_b"] + g["v_w"] @ g["qn_b"]
    qw = g["q_w"] * g["kvn_g"][None, :]
    qb = g["q_b"] + g["q_w"] @ g["kvn_b"]
    f1w = g["f1_w"] * g["n_g"][None, :]
    f1b = g["f1_b"] + g["f1_w"] @ g["n_b"]

    w = {}

    def put_hl(name, arr):
        hi, lo = _hilo(arr * SW)
        w[name + "H"] = np.ascontiguousarray(hi)
        w[name + "L"] = np.ascontiguousarray(lo)

    put_hl("wp", g["llm_proj_w"].T.reshape(KT_L, 128, DIM))
    put_hl("wq", qw.T.reshape(KT_D, 128, DIM))
    put_hl("wk", kw.T.reshape(KT_D, 128, DIM))
    put_hl("wv", vw.T.reshape(KT_D, 128, DIM))
    put_hl("wo", np.ascontiguousarray(
        g["o_w"].T.reshape(HEADS, HEAD_DIM, DIM).transpose(1, 0, 2)))
    put_hl("f1", f1w.T.reshape(KT_D, 128, FF))
    put_hl("f2", g["f2_w"].T.reshape(KT_F, 128, DIM))

    rowv = np.zeros((4, DIM), f32)
    rowv[0] = vb * SW
    rowv[1] = g["o_b"] * SW
    rowv[2] = g["f2_b"] * SW
    rowv[3] = qb * SW
    w["rowv"] = rowv.astype(BF16)

    ppa = np.zeros((128, PP_COLS), dtype=f32)

    def put(col, vec, n):
        ppa[:, col:col + n] = np.asarray(vec, dtype=f32).reshape(n, 128).T

    put(PP_PROJB, g["llm_proj_b"], KT_D)
    put(PP_F1B, f1b, KT_F)
    ppa[:HEAD_DIM, PP_KB:PP_KB + HEADS] = kb.reshape(HEADS, HEAD_DIM).T
    ppa[:, PP_EPS] = EPS
    w["pp"] = ppa

    clip = g["clip_embed"]
    llm = g["llm_embed"]
    in_maps = []
    for c in range(NCORES):
        cs = slice(c * BPC, (c + 1) * BPC)
        m = dict(w)
        embT = llm[cs].reshape(TQ, LLAMA_DIM).T.reshape(KT_L, 128, TQ) * SE
        ehi, elo = _hilo(embT)
        m["embH"] = np.ascontiguousarray(ehi)
        m["embL"] = np.ascontiguousarray(elo)
        m["clipT"] = np.ascontiguousarray(
            clip[cs].reshape(TK, DIM).T.reshape(KT_D, 128, TK)).astype(BF16)
        in_maps.append(m)
    return in_maps


def run(inputs, trace=False):
    nc = _get_nc()
    in_maps = _prep_in_maps(inputs)
    res = bass_utils.run_bass_kernel_spmd(
        nc, in_maps, core_ids=list(range(NCORES)), trace=trace)
    clip = np.asarray(inputs["clip_embed"], dtype=np.float32)
    llm3 = np.empty((B, LL, DIM), dtype=np.float32)
    for c in range(NCORES):
        yT = res.results[c]["outT"].reshape(DIM, TQ)
        llm3[c * BPC:(c + 1) * BPC] = yT.T.reshape(BPC, LL, DIM)
    out = np.concatenate([clip, llm3], axis=1)
    return out, res


def kernel(**inputs):
    out, _ = run(inputs, trace=False)
    return out
